# revision 21
# baseline (speedup 1.0000x reference)
"""Multihead causal attention on 8 TRN2 NeuronCores.

Sharding: core = (batch b, head-group hg): b = core//2, hg = core%2.
Each core gets x[b] (full sequence, [2048, 1024]) plus the weight rows for
its 8 heads (W[hg*512:(hg+1)*512, :]), computes Q/K/V projections and
causal attention for those (batch, head) pairs, and writes Y transposed
as [8, 64, 2048] (head, dh, seq); the host transposes back on gather.

v2 pipeline design (software-pipelined projections + attention):
  - Head pairs g=0..3. Attention for pair g runs interleaved with the
    Q/K projections for pair g+1 (and the V projection, during g=0), so
    ScalarE exp work overlaps projection matmuls instead of idling
    through a separate projection phase.
  - Scores in transposed layout scoresT[k, q] per head; the two heads of
    a pair run as concurrent 64-row PE tiles (tile_position auto-derived
    from base_partition 0/64).
  - Causal masking with no memsets: matmuls / exp activations are
    column-restricted to at-or-below-diagonal ranges, the triangular
    128x128 boundary blocks get one DVE multiply each, and the PV matmul
    reads only valid columns.
  - PV: yy[65, 512] per (head, q-chunk); V carries an appended ones
    column so row 64 accumulates the softmax denominator.  PV for kt
    pair p is emitted after the scores of pair p+1 (one-stage software
    pipeline) so TensorE never waits on the exp.
  - PE warm-up: dummy matmuls at T=0 (during input DMA) flip the HAM
    clock gate to 8/8 before real work arrives.
"""
import numpy as np
import ml_dtypes

import concourse.bass as bass
import concourse.tile as tile
from concourse import bacc, mybir
from concourse.bass_utils import run_bass_kernel_spmd

F32 = mybir.dt.float32
BF16 = mybir.dt.bfloat16
EXP = mybir.ActivationFunctionType.Exp

B, S, D, H, DH = 4, 2048, 1024, 16, 64
N_CORES = 8
H_LOC = 8           # heads per core
D_LOC = H_LOC * DH  # 512: projection output dim per core
N_CT = D // 128     # 8 contraction tiles
N_QT = S // 512     # 4 q-chunks of 512
SCALE = 1.0 / np.sqrt(DH)

_NC_CACHE = {}


def build_nc():
    nc = bacc.Bacc("TRN2", target_bir_lowering=False, debug=False,
                   num_devices=N_CORES)
    # inputs pre-tiled on host as [ct, partition, col]; x.T arrives in
    # column quarters so DMA can be priority-ordered (V projection needs
    # wvt + x columns 0:512 first; x columns 1536:2048 aren't touched
    # until ~60us in)
    xq = [nc.dram_tensor(f"xq{i}", [N_CT, 128, 512], BF16,
                         kind="ExternalInput").ap() for i in range(4)]
    wqt = nc.dram_tensor("wqt", [N_CT, 128, D_LOC], BF16, kind="ExternalInput").ap()
    wkt = nc.dram_tensor("wkt", [N_CT, 128, D_LOC], BF16, kind="ExternalInput").ap()
    wvt = nc.dram_tensor("wvt", [N_CT, 128, D_LOC], BF16, kind="ExternalInput").ap()
    out = nc.dram_tensor("out", [H_LOC, DH, S], F32, kind="ExternalOutput").ap()

    # tri[kk, qq] = 1 iff qq >= kk (valid: query position >= key position)
    tri_np = (np.arange(128)[None, :] >= np.arange(128)[:, None])
    tri_dram = nc.inline_tensor(tri_np.astype(ml_dtypes.bfloat16), name="tri")

    with tile.TileContext(nc) as tc:
        with tc.tile_pool(name="consts", bufs=1) as consts, \
             tc.tile_pool(name="pers", bufs=1) as pers, \
             tc.tile_pool(name="ee", bufs=3) as epool, \
             tc.tile_pool(name="norm", bufs=2) as norm, \
             tc.tile_pool(name="psP", bufs=2, space="PSUM") as psP, \
             tc.tile_pool(name="psS", bufs=1, space="PSUM") as psS, \
             tc.tile_pool(name="psY", bufs=1, space="PSUM") as psY:

            tri = consts.tile([128, 128], BF16)
            nc.sync.dma_start(tri[:], tri_dram.ap())
            warm = consts.tile([128, 128], BF16, name="warm")
            nc.gpsimd.memset(warm[:], 0.0)

            # ---- PE warm-up: bridge the gap from engine start to the
            # first data-ready matmul so HAM flips to 8/8 early ----
            for i in range(12):
                wp = psP.tile([128, 512], F32, tag="pp", name="wp")
                nc.tensor.matmul(wp[:, 0:128], warm[:], warm[:],
                                 start=True, stop=True)

            def dummy_fill(n):
                """Idle-replacement matmuls for Scalar-bound kt pairs:
                keep TensorE active so HAM never re-throttles."""
                for _ in range(n):
                    wp = psP.tile([128, 512], F32, tag="pp", name="wp")
                    nc.tensor.matmul(wp[:, 0:128], warm[:], warm[:],
                                     start=True, stop=True)

            # ---- persistent SBUF tensors ----
            xTq = [pers.tile([128, N_CT * 512], BF16, tag=f"xTq{i}",
                             name=f"xTq{i}") for i in range(4)]
            WT = {w: pers.tile([128, N_CT * D_LOC], BF16, tag=f"W{w}",
                               name=f"W{w}") for w in "qkv"}
            QT = [pers.tile([128, S], BF16, tag=f"QT{i}", name=f"QT{i}")
                  for i in range(4)]
            KT = [pers.tile([128, S], BF16, tag=f"KT{i}", name=f"KT{i}")
                  for i in range(4)]
            VP = [pers.tile([128, H_LOC, DH + 1], BF16, tag=f"VP{i}",
                            name=f"VP{i}") for i in range(16)]

            # ---- input DMA: a few descriptors per tensor (multiple
            # in-flight descriptors engage multiple DMA engines; one big
            # descriptor moves at only ~80 GB/s).  Two queues, priority
            # order: V-projection inputs first, x quarter 3 last. ----
            def dma_in(eng, sbuf_tile, dram, cols):
                for c0 in range(0, N_CT, 4):
                    eng.dma_start(
                        sbuf_tile[:, c0 * cols:(c0 + 4) * cols]
                        .rearrange("p (ct c) -> p ct c", ct=4),
                        dram[c0:c0 + 4].rearrange("ct p c -> p ct c"))

            dma_in(nc.gpsimd, WT["v"], wvt, D_LOC)
            dma_in(nc.sync, xTq[0], xq[0], 512)
            dma_in(nc.sync, xTq[1], xq[1], 512)
            dma_in(nc.gpsimd, WT["q"], wqt, D_LOC)
            dma_in(nc.gpsimd, WT["k"], wkt, D_LOC)
            dma_in(nc.sync, xTq[2], xq[2], 512)
            dma_in(nc.sync, xTq[3], xq[3], 512)

            def xslice(c0, c1, ct):
                """x.T[ct*128:(ct+1)*128, c0:c1] from the quarter tiles."""
                q = c0 // 512
                assert c1 <= (q + 1) * 512
                return xTq[q][:, ct * 512 + c0 - q * 512:
                              ct * 512 + c1 - q * 512]

            # ---- projection bursts (8 accumulating matmuls + 1 cast) ----
            def qk_burst(w, g, qc):
                dst = QT if w == "q" else KT
                pp = psP.tile([128, 512], F32, tag="pp", name="pp")
                for ct in range(N_CT):
                    nc.tensor.matmul(
                        pp[:],
                        WT[w][:, ct * D_LOC + g * 128:ct * D_LOC + (g + 1) * 128],
                        xslice(qc * 512, (qc + 1) * 512, ct),
                        start=(ct == 0), stop=(ct == N_CT - 1))
                nc.vector.tensor_copy(dst[g][:, qc * 512:(qc + 1) * 512], pp[:])

            def v_burst(st):
                pp = psP.tile([128, 512], F32, tag="pp", name="pp")
                for ct in range(N_CT):
                    nc.tensor.matmul(
                        pp[:],
                        xslice(st * 128, (st + 1) * 128, ct),
                        WT["v"][:, ct * D_LOC:(ct + 1) * D_LOC],
                        start=(ct == 0), stop=(ct == N_CT - 1))
                nc.vector.tensor_copy(
                    VP[st][:, :, 0:DH],
                    pp[:].rearrange("p (h d) -> p h d", h=H_LOC))
                nc.vector.memset(VP[st][:, :, DH:DH + 1], 1.0)

            # JIT fillers: fillers[g][qt] is projection work interleaved
            # INTO stage (g, qt)'s kt-pair loop, keeping TensorE dense so
            # the HAM clock gate never re-throttles.  Q/K for q-chunk qc
            # of pair g is produced during stage (g, qc-1) — one stage
            # ahead of first use — and qc=0 during stage (g-1, qt=3).
            fillers = [[[] for _ in range(N_QT)] for _ in range(4)]
            for g in range(4):
                for qt in range(N_QT):
                    fl = fillers[g][qt]
                    if g == 0 and qt < 3:  # V blocks for stage (0, qt+1)
                        fl += [lambda st=st: v_burst(st)
                               for st in range(4 * qt + 4, 4 * qt + 8)]
                    if qt < 3:
                        fl += [lambda w="q", gg=g, qc=qt + 1: qk_burst(w, gg, qc),
                               lambda w="k", gg=g, qc=qt + 1: qk_burst(w, gg, qc)]
                    elif g < 3:
                        fl += [lambda w="q", gg=g + 1: qk_burst(w, gg, 0),
                               lambda w="k", gg=g + 1: qk_burst(w, gg, 0)]

            # ---- attention, software-pipelined over kt pairs ----
            def emit_scores(g, qt, kp):
                """Scores + exp + tri-mask for kt pair (2kp, 2kp+1), both
                heads in one [128, 2048] PSUM tile (head hh at cols
                hh*1024), exp'd with a single wide ACTIVATE (the ~290ns
                per-instruction ramp is the dominant ScalarE overhead).
                Returns the ee tile (bf16 SBUF, [128, 2048])."""
                ps = psS.tile([128, 2048], F32, tag="s", name="s")
                ee = epool.tile([128, 2048], BF16, tag="e", name="e")
                offs = [max(0, (2 * kp + j) * 128 - qt * 512) for j in (0, 1)]
                q0 = qt * 512
                # scores: j-outer, hh-inner so consecutive LDWEIGHTS
                # alternate 64-row PE tiles and overlap the other matmul
                for j in (0, 1):
                    kt = 2 * kp + j
                    for hh in range(2):
                        rows = slice(hh * 64, hh * 64 + 64)
                        c0 = hh * 1024 + j * 512 + offs[j]
                        nc.tensor.matmul(
                            ps[:, c0:hh * 1024 + (j + 1) * 512],
                            KT[g][rows, kt * 128:(kt + 1) * 128],
                            QT[g][rows, q0 + offs[j]:q0 + 512],
                            start=True, stop=True)
                diag = (2 * kp >= 4 * qt)
                if diag:
                    # exact per-block activations (no stale-PSUM reads)
                    for hh in range(2):
                        for j in (0, 1):
                            c0 = hh * 1024 + j * 512 + offs[j]
                            nc.scalar.activation(
                                ee[:, c0:hh * 1024 + (j + 1) * 512],
                                ps[:, c0:hh * 1024 + (j + 1) * 512],
                                EXP, scale=SCALE)
                else:
                    # fully-written tile: one wide act (saves the ~290ns
                    # ACTIVATE ramp vs per-head acts)
                    nc.scalar.activation(ee[:], ps[:], EXP, scale=SCALE)
                if diag:
                    for hh in range(2):
                        for j in (0, 1):
                            c0 = hh * 1024 + j * 512 + offs[j]
                            nc.vector.tensor_mul(
                                ee[:, c0:c0 + 128], ee[:, c0:c0 + 128],
                                tri[:])
                return ee

            def emit_pv(g, qt, kp, ee, yy, n_kt):
                for hh in range(2):
                    for j in (0, 1):
                        kt = 2 * kp + j
                        off = max(0, kt * 128 - qt * 512)
                        nc.tensor.matmul(
                            yy[hh][:, off:512],
                            VP[kt][:, 2 * g + hh, :],
                            ee[:, hh * 1024 + j * 512 + off:
                               hh * 1024 + (j + 1) * 512],
                            start=(kt == 0), stop=(kt == n_kt - 1))

            # startup projections: V[0:4] + Q/K q-chunk 0 for pair 0
            for st in range(4):
                v_burst(st)
            qk_burst("q", 0, 0)
            qk_burst("k", 0, 0)

            for g in range(4):
                for qt in range(N_QT):
                    fl = fillers[g][qt]
                    n_kt = 4 * (qt + 1)
                    n_kp = n_kt // 2
                    q0 = qt * 512
                    yy = [psY.tile([DH + 1, 512], F32, tag=f"y{hh}",
                                   name=f"y{hh}") for hh in range(2)]
                    pend = None  # (kp, ee) awaiting PV emission
                    fi = 0  # fillers emitted so far
                    for kp in range(n_kp):
                        ee = emit_scores(g, qt, kp)
                        if pend is not None:
                            emit_pv(g, qt, pend[0], pend[1], yy, n_kt)
                        pend = (kp, ee)
                        # interleave a proportional share of the fillers;
                        # in pair 3 (no projection work left) pad filler-
                        # less kt pairs with dummy matmuls to hold HAM
                        want = (len(fl) * (kp + 1)) // n_kp
                        if fi == want and g == 3:
                            dummy_fill(2)
                        while fi < want:
                            fl[fi]()
                            fi += 1
                    while fi < len(fl):
                        fl[fi]()
                        fi += 1
                    emit_pv(g, qt, pend[0], pend[1], yy, n_kt)
                    for hh in range(2):
                        den = norm.tile([1, 512], F32, tag="den")
                        nc.vector.tensor_copy(den[:], yy[hh][DH:DH + 1, :])
                        rd = norm.tile([1, 512], F32, tag="rd")
                        nc.vector.reciprocal_approx_fast(rd[:], den[:])
                        rdb = norm.tile([DH, 512], F32, tag="rdb")
                        nc.gpsimd.partition_broadcast(rdb[:], rd[:])
                        yn = norm.tile([DH, 512], F32, tag="yn")
                        nc.vector.tensor_mul(yn[:], yy[hh][0:DH, :], rdb[:])
                        nc.sync.dma_start(out[2 * g + hh, :, q0:q0 + 512],
                                          yn[:])
    nc.compile()
    return nc


def get_nc():
    if "nc" not in _NC_CACHE:
        _NC_CACHE["nc"] = build_nc()
    return _NC_CACHE["nc"]


def _tile_ct(a):
    """[D, C] -> contiguous [D//128, 128, C]."""
    return np.ascontiguousarray(a.reshape(N_CT, 128, a.shape[1]))


def make_in_maps(x, W_q, W_k, W_v):
    in_maps = []
    for core in range(N_CORES):
        b, hg = core // 2, core % 2
        rows = slice(hg * D_LOC, (hg + 1) * D_LOC)
        bf = ml_dtypes.bfloat16
        xt = np.asarray(x[b], dtype=np.float32).T.astype(bf)  # [D, S]
        m = {f"xq{i}": _tile_ct(np.ascontiguousarray(xt[:, i * 512:(i + 1) * 512]))
             for i in range(4)}
        m["wqt"] = _tile_ct(np.asarray(W_q[rows], dtype=np.float32).T.astype(bf))
        m["wkt"] = _tile_ct(np.asarray(W_k[rows], dtype=np.float32).T.astype(bf))
        m["wvt"] = _tile_ct(np.asarray(W_v[rows], dtype=np.float32).T.astype(bf))
        in_maps.append(m)
    return in_maps


def assemble(results):
    Y = np.empty((B, H, S, DH), dtype=np.float32)
    for core in range(N_CORES):
        b, hg = core // 2, core % 2
        yc = results[core]["out"]  # [H_LOC, DH, S]
        Y[b, hg * H_LOC:(hg + 1) * H_LOC] = yc.transpose(0, 2, 1)
    return Y


def kernel(x, W_q, W_k, W_v):
    nc = get_nc()
    in_maps = make_in_maps(x, W_q, W_k, W_v)
    res = run_bass_kernel_spmd(nc, in_maps, list(range(N_CORES)))
    return assemble(res.results)


# revision 23
# speedup vs baseline: 1.1242x; 1.1242x over previous
"""Multihead causal attention on 8 TRN2 NeuronCores.

Sharding: core = (batch b, head-group hg): b = core//2, hg = core%2.
Each core gets x[b] (full sequence, [2048, 1024]) plus the weight rows for
its 8 heads (W[hg*512:(hg+1)*512, :]), computes Q/K/V projections and
causal attention for those (batch, head) pairs, and writes Y transposed
as [8, 64, 2048] (head, dh, seq); the host transposes back on gather.

v2 pipeline design (software-pipelined projections + attention):
  - Head pairs g=0..3. Attention for pair g runs interleaved with the
    Q/K projections for pair g+1 (and the V projection, during g=0), so
    ScalarE exp work overlaps projection matmuls instead of idling
    through a separate projection phase.
  - Scores in transposed layout scoresT[k, q] per head; the two heads of
    a pair run as concurrent 64-row PE tiles (tile_position auto-derived
    from base_partition 0/64).
  - Causal masking with no memsets: matmuls / exp activations are
    column-restricted to at-or-below-diagonal ranges, the triangular
    128x128 boundary blocks get one DVE multiply each, and the PV matmul
    reads only valid columns.
  - PV: yy[65, 512] per (head, q-chunk); V carries an appended ones
    column so row 64 accumulates the softmax denominator.  PV for kt
    pair p is emitted after the scores of pair p+1 (one-stage software
    pipeline) so TensorE never waits on the exp.
  - PE warm-up: dummy matmuls at T=0 (during input DMA) flip the HAM
    clock gate to 8/8 before real work arrives.
"""
import numpy as np
import ml_dtypes

import concourse.bass as bass
import concourse.tile as tile
from concourse import bacc, mybir
from concourse.bass_utils import run_bass_kernel_spmd

F32 = mybir.dt.float32
BF16 = mybir.dt.bfloat16
EXP = mybir.ActivationFunctionType.Exp

B, S, D, H, DH = 4, 2048, 1024, 16, 64
N_CORES = 8
H_LOC = 8           # heads per core
D_LOC = H_LOC * DH  # 512: projection output dim per core
N_CT = D // 128     # 8 contraction tiles
N_QT = S // 512     # 4 q-chunks of 512
SCALE = 1.0 / np.sqrt(DH)

_NC_CACHE = {}


def build_nc():
    nc = bacc.Bacc("TRN2", target_bir_lowering=False, debug=False,
                   num_devices=N_CORES)
    # inputs pre-tiled on host as [ct, partition, col]; x.T arrives in
    # column quarters so DMA can be priority-ordered (V projection needs
    # wvt + x columns 0:512 first; x columns 1536:2048 aren't touched
    # until ~60us in)
    xq = [nc.dram_tensor(f"xq{i}", [N_CT, 128, 512], BF16,
                         kind="ExternalInput").ap() for i in range(4)]
    wqt = nc.dram_tensor("wqt", [N_CT, 128, D_LOC], BF16, kind="ExternalInput").ap()
    wkt = nc.dram_tensor("wkt", [N_CT, 128, D_LOC], BF16, kind="ExternalInput").ap()
    wvt = nc.dram_tensor("wvt", [N_CT, 128, D_LOC], BF16, kind="ExternalInput").ap()
    out = nc.dram_tensor("out", [H_LOC, DH, S], F32, kind="ExternalOutput").ap()

    # tri[kk, qq] = 1 iff qq >= kk (valid: query position >= key position)
    tri_np = (np.arange(128)[None, :] >= np.arange(128)[:, None])
    tri_dram = nc.inline_tensor(tri_np.astype(ml_dtypes.bfloat16), name="tri")

    with tile.TileContext(nc) as tc:
        with tc.tile_pool(name="consts", bufs=1) as consts, \
             tc.tile_pool(name="pers", bufs=1) as pers, \
             tc.tile_pool(name="ee", bufs=3) as epool, \
             tc.tile_pool(name="norm", bufs=2) as norm, \
             tc.tile_pool(name="psP", bufs=2, space="PSUM") as psP, \
             tc.tile_pool(name="psS", bufs=1, space="PSUM") as psS, \
             tc.tile_pool(name="psY", bufs=1, space="PSUM") as psY:

            tri = consts.tile([128, 128], BF16)
            nc.sync.dma_start(tri[:], tri_dram.ap())
            warm = consts.tile([128, 128], BF16, name="warm")
            nc.gpsimd.memset(warm[:], 0.0)

            # ---- PE warm-up: bridge the gap from engine start to the
            # first data-ready matmul so HAM flips to 8/8 early ----
            for i in range(12):
                wp = psP.tile([128, 512], F32, tag="pp", name="wp")
                nc.tensor.matmul(wp[:, 0:128], warm[:], warm[:],
                                 start=True, stop=True)

            def dummy_fill(n):
                """Idle-replacement matmuls for Scalar-bound kt pairs:
                keep TensorE active so HAM never re-throttles."""
                for _ in range(n):
                    wp = psP.tile([128, 512], F32, tag="pp", name="wp")
                    nc.tensor.matmul(wp[:, 0:128], warm[:], warm[:],
                                     start=True, stop=True)

            # ---- persistent SBUF tensors ----
            xTq = [pers.tile([128, N_CT * 512], BF16, tag=f"xTq{i}",
                             name=f"xTq{i}") for i in range(4)]
            WT = {w: pers.tile([128, N_CT * D_LOC], BF16, tag=f"W{w}",
                               name=f"W{w}") for w in "qkv"}
            QT = [pers.tile([128, S], BF16, tag=f"QT{i}", name=f"QT{i}")
                  for i in range(4)]
            KT = [pers.tile([128, S], BF16, tag=f"KT{i}", name=f"KT{i}")
                  for i in range(4)]
            VP = [pers.tile([128, H_LOC, DH + 1], BF16, tag=f"VP{i}",
                            name=f"VP{i}") for i in range(16)]

            # ---- input DMA: a few descriptors per tensor (multiple
            # in-flight descriptors engage multiple DMA engines; one big
            # descriptor moves at only ~80 GB/s).  Two queues, priority
            # order: V-projection inputs first, x quarter 3 last. ----
            def dma_in(eng, sbuf_tile, dram, cols):
                for c0 in range(0, N_CT, 4):
                    eng.dma_start(
                        sbuf_tile[:, c0 * cols:(c0 + 4) * cols]
                        .rearrange("p (ct c) -> p ct c", ct=4),
                        dram[c0:c0 + 4].rearrange("ct p c -> p ct c"))

            dma_in(nc.gpsimd, WT["v"], wvt, D_LOC)
            dma_in(nc.sync, xTq[0], xq[0], 512)
            dma_in(nc.sync, xTq[1], xq[1], 512)
            dma_in(nc.gpsimd, WT["q"], wqt, D_LOC)
            dma_in(nc.gpsimd, WT["k"], wkt, D_LOC)
            dma_in(nc.sync, xTq[2], xq[2], 512)
            dma_in(nc.sync, xTq[3], xq[3], 512)

            def xslice(c0, c1, ct):
                """x.T[ct*128:(ct+1)*128, c0:c1] from the quarter tiles."""
                q = c0 // 512
                assert c1 <= (q + 1) * 512
                return xTq[q][:, ct * 512 + c0 - q * 512:
                              ct * 512 + c1 - q * 512]

            # ---- projection bursts (8 accumulating matmuls + 1 cast) ----
            def qk_burst(w, g, qc):
                dst = QT if w == "q" else KT
                pp = psP.tile([128, 512], F32, tag="pp", name="pp")
                for ct in range(N_CT):
                    nc.tensor.matmul(
                        pp[:],
                        WT[w][:, ct * D_LOC + g * 128:ct * D_LOC + (g + 1) * 128],
                        xslice(qc * 512, (qc + 1) * 512, ct),
                        start=(ct == 0), stop=(ct == N_CT - 1))
                nc.vector.tensor_copy(dst[g][:, qc * 512:(qc + 1) * 512], pp[:])

            def v_burst(st):
                pp = psP.tile([128, 512], F32, tag="pp", name="pp")
                for ct in range(N_CT):
                    nc.tensor.matmul(
                        pp[:],
                        xslice(st * 128, (st + 1) * 128, ct),
                        WT["v"][:, ct * D_LOC:(ct + 1) * D_LOC],
                        start=(ct == 0), stop=(ct == N_CT - 1))
                nc.vector.tensor_copy(
                    VP[st][:, :, 0:DH],
                    pp[:].rearrange("p (h d) -> p h d", h=H_LOC))
                nc.vector.memset(VP[st][:, :, DH:DH + 1], 1.0)

            # JIT fillers: fillers[g][qt] is projection work interleaved
            # INTO stage (g, qt)'s kt-pair loop, keeping TensorE dense so
            # the HAM clock gate never re-throttles.  Q/K for q-chunk qc
            # of pair g is produced during stage (g, qc-1) — one stage
            # ahead of first use — and qc=0 during stage (g-1, qt=3).
            fillers = [[[] for _ in range(N_QT)] for _ in range(4)]
            for g in range(4):
                for qt in range(N_QT):
                    fl = fillers[g][qt]
                    if g == 0 and qt < 3:  # V blocks for stage (0, qt+1)
                        fl += [lambda st=st: v_burst(st)
                               for st in range(4 * qt + 4, 4 * qt + 8)]
                    if qt < 3:
                        fl += [lambda w="q", gg=g, qc=qt + 1: qk_burst(w, gg, qc),
                               lambda w="k", gg=g, qc=qt + 1: qk_burst(w, gg, qc)]
                    elif g < 3:
                        fl += [lambda w="q", gg=g + 1: qk_burst(w, gg, 0),
                               lambda w="k", gg=g + 1: qk_burst(w, gg, 0)]

            # ---- attention, software-pipelined over kt pairs ----
            def emit_scores(g, qt, kp):
                """Scores + exp + tri-mask for kt pair (2kp, 2kp+1).
                Separate per-head PSUM tiles / activations keep the
                slot-reuse chain per head (sMM(kp+1) of head hh waits only
                act(kp) of hh) — a merged tile serializes Scalar<->Tensor
                and makes the HAM clock gate oscillate.
                Returns the ee tiles (bf16 SBUF, [128, 1024])."""
                ps = [psS.tile([128, 1024], F32, tag=f"s{hh}", name=f"s{hh}")
                      for hh in range(2)]
                ee = [epool.tile([128, 1024], BF16, tag=f"e{hh}",
                                 name=f"e{hh}") for hh in range(2)]
                offs = [max(0, (2 * kp + j) * 128 - qt * 512) for j in (0, 1)]
                q0 = qt * 512
                # scores: j-outer, hh-inner so consecutive LDWEIGHTS
                # alternate 64-row PE tiles and overlap the other matmul
                for j in (0, 1):
                    kt = 2 * kp + j
                    for hh in range(2):
                        rows = slice(hh * 64, hh * 64 + 64)
                        nc.tensor.matmul(
                            ps[hh][:, j * 512 + offs[j]:(j + 1) * 512],
                            KT[g][rows, kt * 128:(kt + 1) * 128],
                            QT[g][rows, q0 + offs[j]:q0 + 512],
                            start=True, stop=True)
                diag = (2 * kp >= 4 * qt)
                for hh in range(2):
                    if not diag:
                        nc.scalar.activation(ee[hh][:], ps[hh][:], EXP,
                                             scale=SCALE)
                    else:
                        for j in (0, 1):
                            c0 = j * 512 + offs[j]
                            nc.scalar.activation(
                                ee[hh][:, c0:(j + 1) * 512],
                                ps[hh][:, c0:(j + 1) * 512], EXP, scale=SCALE)
                if diag:
                    for hh in range(2):
                        for j in (0, 1):
                            c0 = j * 512 + offs[j]
                            nc.vector.tensor_mul(
                                ee[hh][:, c0:c0 + 128],
                                ee[hh][:, c0:c0 + 128], tri[:])
                return ee

            def emit_pv(g, qt, kp, ee, yy, n_kt):
                for hh in range(2):
                    for j in (0, 1):
                        kt = 2 * kp + j
                        off = max(0, kt * 128 - qt * 512)
                        nc.tensor.matmul(
                            yy[hh][:, off:512],
                            VP[kt][:, 2 * g + hh, :],
                            ee[hh][:, j * 512 + off:(j + 1) * 512],
                            start=(kt == 0), stop=(kt == n_kt - 1))

            # startup projections: V[0:4] + Q/K q-chunk 0 for pair 0
            for st in range(4):
                v_burst(st)
            qk_burst("q", 0, 0)
            qk_burst("k", 0, 0)

            for g in range(4):
                for qt in range(N_QT):
                    fl = fillers[g][qt]
                    n_kt = 4 * (qt + 1)
                    n_kp = n_kt // 2
                    q0 = qt * 512
                    yy = [psY.tile([DH + 1, 512], F32, tag=f"y{hh}",
                                   name=f"y{hh}") for hh in range(2)]
                    pend = None  # (kp, ee) awaiting PV emission
                    fi = 0  # fillers emitted so far
                    for kp in range(n_kp):
                        ee = emit_scores(g, qt, kp)
                        if pend is not None:
                            emit_pv(g, qt, pend[0], pend[1], yy, n_kt)
                        pend = (kp, ee)
                        # interleave a proportional share of the fillers;
                        # in pair 3 (no projection work left) pad filler-
                        # less kt pairs with dummy matmuls to hold HAM
                        want = (len(fl) * (kp + 1)) // n_kp
                        if fi == want and (g == 3 or (g == 2 and qt == 3)):
                            dummy_fill(2)
                        while fi < want:
                            fl[fi]()
                            fi += 1
                    while fi < len(fl):
                        fl[fi]()
                        fi += 1
                    emit_pv(g, qt, pend[0], pend[1], yy, n_kt)
                    for hh in range(2):
                        den = norm.tile([1, 512], F32, tag="den")
                        nc.vector.tensor_copy(den[:], yy[hh][DH:DH + 1, :])
                        rd = norm.tile([1, 512], F32, tag="rd")
                        nc.vector.reciprocal_approx_fast(rd[:], den[:])
                        rdb = norm.tile([DH, 512], F32, tag="rdb")
                        nc.gpsimd.partition_broadcast(rdb[:], rd[:])
                        yn = norm.tile([DH, 512], F32, tag="yn")
                        nc.vector.tensor_mul(yn[:], yy[hh][0:DH, :], rdb[:])
                        nc.sync.dma_start(out[2 * g + hh, :, q0:q0 + 512],
                                          yn[:])
    nc.compile()
    return nc


def get_nc():
    if "nc" not in _NC_CACHE:
        _NC_CACHE["nc"] = build_nc()
    return _NC_CACHE["nc"]


def _tile_ct(a):
    """[D, C] -> contiguous [D//128, 128, C]."""
    return np.ascontiguousarray(a.reshape(N_CT, 128, a.shape[1]))


def make_in_maps(x, W_q, W_k, W_v):
    in_maps = []
    for core in range(N_CORES):
        b, hg = core // 2, core % 2
        rows = slice(hg * D_LOC, (hg + 1) * D_LOC)
        bf = ml_dtypes.bfloat16
        xt = np.asarray(x[b], dtype=np.float32).T.astype(bf)  # [D, S]
        m = {f"xq{i}": _tile_ct(np.ascontiguousarray(xt[:, i * 512:(i + 1) * 512]))
             for i in range(4)}
        m["wqt"] = _tile_ct(np.asarray(W_q[rows], dtype=np.float32).T.astype(bf))
        m["wkt"] = _tile_ct(np.asarray(W_k[rows], dtype=np.float32).T.astype(bf))
        m["wvt"] = _tile_ct(np.asarray(W_v[rows], dtype=np.float32).T.astype(bf))
        in_maps.append(m)
    return in_maps


def assemble(results):
    Y = np.empty((B, H, S, DH), dtype=np.float32)
    for core in range(N_CORES):
        b, hg = core // 2, core % 2
        yc = results[core]["out"]  # [H_LOC, DH, S]
        Y[b, hg * H_LOC:(hg + 1) * H_LOC] = yc.transpose(0, 2, 1)
    return Y


def kernel(x, W_q, W_k, W_v):
    nc = get_nc()
    in_maps = make_in_maps(x, W_q, W_k, W_v)
    res = run_bass_kernel_spmd(nc, in_maps, list(range(N_CORES)))
    return assemble(res.results)
